# revision 28
# baseline (speedup 1.0000x reference)
"""Single-head causal self-attention on 8 TRN2 NeuronCores.

Problem: x [8, 4096, 1024] f32, Wq/Wk/Wv [1024, 128] f32
  q/k/v = x @ W*;  out = softmax(causal(q k^T / sqrt(128))) @ v   -> [8, 4096, 128] f32

Sharding: data-parallel over batch B=8 -> one batch element per core, weights
replicated. No collectives.

Per-core plan (T=4096, C=1024, D=128):
  - x^T and W staged in fp8 (e4m3, W prescaled x64 to dodge subnormals);
    QKV projections run as fp8 DoubleRow matmuls (K=256/pass, ~1.9x bf16)
  - q/k copied to SBUF bf16 (descale 1/64); scores S^T = K^T.Q stay bf16
    (K=128 contraction gains nothing from fp8)
  - P = exp(S/sqrt(128)) written directly as fp8: ScalarE uses the Exp
    activation; the DVE share uses a Schraudolph-style one-pass
    tensor_scalar (bits = round(s*8*log2e + 55.54) -> int8 == e4m3 bits),
    accuracy ~= fp8-quantized exact exp. Work is split between ACT/DVE by
    a build-time load balancer.
  - PV accumulates acc[tq,129] over s-tile PAIRS with fp8 DoubleRow
    (V_aug pairs [128,2,129] carry a ones column -> col 128 = sum(exp));
    diagonal-adjacent odd tails run as plain fp8 matmuls
  - the softmax division happens on HOST (kernel returns [T,129] of
    numerator|denominator) - saves all on-chip reciprocal/mul work
  - rows 0..127 (tq tile 0) use a bf16 PV path: with few causal keys the
    fp8 V quantization error does not average out
  - diagonal 128x128 blocks masked on GpSimd (affine_select, s <= tq)
"""

import numpy as np
import ml_dtypes

B, T, C, D = 8, 4096, 1024, 128
N_CORES = 8
GQ = 1024              # tq group width
N_G = T // GQ          # 4 tq groups
N_JP = 4               # c-chunk pairs (C = 4 * 256)
INV_SQRT_D = 1.0 / float(np.sqrt(D))
W_PRESCALE = 64.0
LOG2E = float(np.log2(np.e))
SCH_A = 8.0 * LOG2E * INV_SQRT_D   # schraudolph scale on raw score psum
SCH_B = 56.0 - 0.46                # fitted offset (DVE convert rounds-to-nearest)
FP8NP = ml_dtypes.float8_e4m3

PASS_TILES = [(0, 1, 2), (3, 4, 5), (6, 7)]
PASS_TILES_G3 = [(0, 1, 2, 3), (4, 5), (6, 7)]
N_PREFETCH = {1: 2, 2: 5, 3: 12}   # js of group g prefetched into g-1's passes

_CACHE = {}


class Chooser:
    """Build-time ACT/DVE load balancer for psum-reading elementwise ops."""

    def __init__(self, nc, mybir):
        self.nc = nc
        self.mybir = mybir
        self.busy = {"act": 0.0, "dve": 0.0}

    def _pick(self, cols, act_extra=0.0):
        ca = self.busy["act"] + 0.833 * cols + 260 + act_extra
        cd = self.busy["dve"] + 1.042 * cols + 145
        return "act" if ca <= cd else "dve"

    def exp_fp8(self, out_ap, in_ap, cols):
        eng = self._pick(cols)
        if eng == "act":
            self.nc.scalar.activation(out_ap, in_ap,
                                      self.mybir.ActivationFunctionType.Exp,
                                      scale=INV_SQRT_D)
            self.busy["act"] += 0.833 * cols + 260
        else:
            self.nc.vector.tensor_scalar(out_ap.bitcast(self.mybir.dt.int8),
                                         in_ap, SCH_A, SCH_B,
                                         self.mybir.AluOpType.mult,
                                         self.mybir.AluOpType.add)
            self.busy["dve"] += 1.042 * cols + 145
        return eng

    def copy_scaled(self, out_ap, in_ap, cols, scale):
        eng = self._pick(cols)
        if eng == "act":
            self.nc.scalar.activation(out_ap, in_ap,
                                      self.mybir.ActivationFunctionType.Copy,
                                      scale=scale)
            self.busy["act"] += 0.833 * cols + 260
        else:
            self.nc.vector.tensor_scalar(out_ap, in_ap, scale, None,
                                         self.mybir.AluOpType.mult)
            self.busy["dve"] += 1.042 * cols + 145
        return eng


def _build_nc():
    import concourse.tile as tile
    from concourse import bacc, mybir

    f32 = mybir.dt.float32
    bf16 = mybir.dt.bfloat16
    fp8 = mybir.dt.float8e4
    DR = mybir.MatmulPerfMode.DoubleRow

    nc = bacc.Bacc(None, target_bir_lowering=False)
    # xt[jp*N_G + g] is [128, 2*GQ]: c-pair jp, group g, slots (2jp, 2jp+1)
    xt_d = nc.declare_dram_parameter("xt", [N_JP * N_G, 128, 2 * GQ], fp8,
                                     isOutput=False)
    wq_d = nc.declare_dram_parameter("wq", [128, 8 * D], fp8, isOutput=False)
    wk_d = nc.declare_dram_parameter("wk", [128, 8 * D], fp8, isOutput=False)
    wv_d = nc.declare_dram_parameter("wv", [128, 8 * D], fp8, isOutput=False)
    # bf16 copies for the first 128 rows of Q^T/K^T: with few causal keys the
    # fp8 projection noise does not average out of the softmax
    xt0b_d = nc.declare_dram_parameter("xt0b", [128, 8 * 128], bf16,
                                       isOutput=False)
    wqb_d = nc.declare_dram_parameter("wqb", [128, 8 * D], bf16, isOutput=False)
    wkb_d = nc.declare_dram_parameter("wkb", [128, 8 * D], bf16, isOutput=False)
    wvb_d = nc.declare_dram_parameter("wvb", [128, 8 * D], bf16, isOutput=False)
    out_d = nc.declare_dram_parameter("out", [T, D + 1], f32, isOutput=True)

    with tile.TileContext(nc) as tc:
        with (
            tc.tile_pool(name="consts", bufs=1) as consts,
            tc.tile_pool(name="xt", bufs=1) as xt_pool,
            tc.tile_pool(name="qk", bufs=1) as qk_pool,
            tc.tile_pool(name="vaug", bufs=1) as vaug_pool,
            tc.tile_pool(name="p", bufs=1) as p_pool,
            tc.tile_pool(name="osb", bufs=2) as o_pool,
            tc.tile_pool(name="psS", bufs=2, space="PSUM") as psS,
            tc.tile_pool(name="psA", bufs=3, space="PSUM") as psA,
            tc.tile_pool(name="psP", bufs=1, space="PSUM") as psP,
        ):
            ch = Chooser(nc, mybir)

            wq_sb = consts.tile([128, 8, D], fp8, tag="wq")
            wk_sb = consts.tile([128, 8, D], fp8, tag="wk")
            wv_sb = consts.tile([128, 8, D], fp8, tag="wv")

            xt_sb = [[None] * N_G for _ in range(N_JP)]

            def xt_dma(jp, g, eng=None):
                t_ = xt_pool.tile([128, 2, GQ], fp8, tag=f"xt_{jp}_{g}",
                                  name=f"xt_{jp}_{g}")
                (eng or nc.sync).dma_start(t_[:].rearrange("p a b -> p (a b)"),
                                           xt_d[jp * N_G + g])
                xt_sb[jp][g] = t_

            xt0b_sb = consts.tile([128, 8, 128], bf16, tag="xt0b")
            wqb_sb = consts.tile([128, 8, D], bf16, tag="wqb")
            wkb_sb = consts.tile([128, 8, D], bf16, tag="wkb")
            wvb_sb = consts.tile([128, 8, D], bf16, tag="wvb")

            # startup DMAs fan out over four engine queues so group-0 inputs
            # land ~4x sooner; bf16 fixup inputs go first (PE warm-up work)
            nc.sync.dma_start(wqb_sb[:].rearrange("p a b -> p (a b)"), wqb_d[:])
            nc.scalar.dma_start(wkb_sb[:].rearrange("p a b -> p (a b)"), wkb_d[:])
            nc.gpsimd.dma_start(xt0b_sb[:].rearrange("p a b -> p (a b)"), xt0b_d[:])
            nc.sync.dma_start(wvb_sb[:].rearrange("p a b -> p (a b)"), wvb_d[:])
            nc.scalar.dma_start(wq_sb[:].rearrange("p a b -> p (a b)"), wq_d[:])
            nc.gpsimd.dma_start(wk_sb[:].rearrange("p a b -> p (a b)"), wk_d[:])
            nc.sync.dma_start(wv_sb[:].rearrange("p a b -> p (a b)"), wv_d[:])
            xt_dma(0, 0, nc.scalar)
            xt_dma(1, 0, nc.gpsimd)
            xt_dma(2, 0, nc.sync)
            xt_dma(3, 0, nc.scalar)
            for g in range(1, N_G):
                for jp in range(N_JP):
                    xt_dma(jp, g)

            qT = [None] * N_G          # [128, GQ] bf16
            kT = [None] * N_G
            v_pair = [None] * (T // 256)   # [128, 2, D+1] fp8, ones col at 128
            v0_bf = consts.tile([128, D + 1], bf16, tag="v0bf")
            p00_bf = consts.tile([128, 128], bf16, tag="p00bf")

            def qk_half_units(g):
                """Q^T/K^T for group g: 4 units (q/k x half), each 4 DR matmuls.

                For g==0 h==0 only cols 128:512 are produced - cols 0:128 come
                from the bf16 fixup units (which run first)."""
                units = []
                for w_sb, dest_list, nm in ((wq_sb, qT, "q"), (wk_sb, kT, "k")):
                    for h in range(2):
                        def unit(g=g, w_sb=w_sb, dest_list=dest_list, nm=nm, h=h,
                                 pool=None):
                            lo = 128 if (g == 0 and h == 0) else 0
                            hs = slice(h * 512 + lo, (h + 1) * 512)
                            n_cols = 512 - lo
                            pp = pool if pool is not None else psP
                            tg = "psP" if pp is psP else "psA"
                            ps = pp.tile([128, 512], f32, tag=tg,
                                         name=f"ps{nm}_{g}_{h}")
                            for jp in range(N_JP):
                                nc.tensor.matmul(
                                    ps[:, 0:n_cols], w_sb[:, 2 * jp:2 * jp + 2, :],
                                    xt_sb[jp][g][:, :, hs],
                                    start=(jp == 0), stop=(jp == N_JP - 1),
                                    perf_mode=DR)
                            if dest_list[g] is None:
                                dest_list[g] = qk_pool.tile(
                                    [128, GQ], bf16, tag=f"{nm}_{g}",
                                    name=f"{nm}_{g}")
                            ch.copy_scaled(dest_list[g][:, hs], ps[:, 0:n_cols],
                                           n_cols, 1.0 / W_PRESCALE)
                        units.append(unit)
                return units

            def v_unit(i, pool=None):
                def unit(i=i, pool=pool):
                    g, off = i // 8, (i % 8) * 128
                    pp = pool if pool is not None else psP
                    tg = "psP" if pp is psP else "psA"
                    psv = pp.tile([128, 512], f32, tag=tg, name=f"psv_{i}")
                    for jp in range(N_JP):
                        nc.tensor.matmul(
                            psv[:, 0:D], xt_sb[jp][g][:, :, off:off + 128],
                            wv_sb[:, 2 * jp:2 * jp + 2, :],
                            start=(jp == 0), stop=(jp == N_JP - 1),
                            perf_mode=DR)
                    a, sl = i // 2, i % 2
                    if v_pair[a] is None:
                        v_pair[a] = vaug_pool.tile([128, 2, D + 1], fp8,
                                                   tag=f"v_{a}", name=f"v_{a}")
                    ch.copy_scaled(v_pair[a][:, sl, 0:D], psv[:, 0:D], D,
                                   1.0 / W_PRESCALE)
                    nc.gpsimd.memset(v_pair[a][:, sl, D:D + 1], 1.0)
                return unit

            def score_exp(g, js, dest):
                """dest: per-group dict a -> [128, 2, GQ] fp8 tile."""
                off = max(0, (js - 8 * g)) * 128
                pss = psS.tile([128, GQ], f32, tag="psS", name=f"pss_{g}_{js}")
                for h in range(2):
                    lo = max(off, h * 512)
                    if (h + 1) * 512 > lo:
                        nc.tensor.matmul(pss[:, lo:(h + 1) * 512],
                                         kT[js // 8][:, (js % 8) * 128:(js % 8 + 1) * 128],
                                         qT[g][:, lo:(h + 1) * 512],
                                         start=True, stop=True)
                a, sl = js // 2, js % 2
                if a not in dest:
                    dest[a] = p_pool.tile([128, 2, GQ], fp8, tag=f"p_{a}",
                                          bufs=(2 if a < 6 else 1),
                                          name=f"p_{g}_{a}")
                p_t = dest[a]
                ch.exp_fp8(p_t[:, sl, off:GQ], pss[:, off:GQ], GQ - off)
                if js >= 8 * g:
                    nc.gpsimd.affine_select(
                        out=p_t[:, sl, off:off + 128].bitcast(mybir.dt.int8),
                        in_=p_t[:, sl, off:off + 128].bitcast(mybir.dt.int8),
                        compare_op=mybir.AluOpType.is_ge,
                        fill=0.0, base=0, pattern=[[1, 128]],
                        channel_multiplier=-1)
                if g == 0 and js == 0:
                    # bf16 P for rows 0..127 (tile-0 special case)
                    nc.scalar.activation(p00_bf[:], pss[:, 0:128],
                                         mybir.ActivationFunctionType.Exp,
                                         scale=INV_SQRT_D)
                    ch.busy["act"] += 0.833 * 128 + 260
                    nc.gpsimd.affine_select(
                        out=p00_bf[:], in_=p00_bf[:],
                        compare_op=mybir.AluOpType.is_ge,
                        fill=0.0, base=0, pattern=[[1, 128]],
                        channel_multiplier=-1)
                return p_t

            def pv_emit(g, a_or_js, accs, tiles, started, mode, p_cur):
                """Emit PV matmuls for pair index a (mode='pair': both slots
                valid) or diagonal single js (mode='single')."""
                for t in tiles:
                    T_t = 8 * g + t
                    if mode == "pair":
                        a = a_or_js
                        if 2 * a + 1 > T_t:
                            continue
                        last = (T_t - 1) // 2 == a and T_t % 2 == 1
                        if g == 0 and t == 0:
                            continue  # tile 0 handled by bf16 special case
                        nc.tensor.matmul(
                            accs[t][:, 0:D + 1],
                            p_cur[a][:, :, t * 128:(t + 1) * 128],
                            v_pair[a][:],
                            start=(t, g) not in started and not started.add((t, g)),
                            stop=last, perf_mode=DR)
                    else:
                        js = a_or_js
                        if T_t != js:
                            continue
                        if g == 0 and t == 0:
                            nc.tensor.matmul(accs[t][:, 0:D + 1], p00_bf[:],
                                             v0_bf[:], start=True, stop=True)
                        else:
                            a, sl = js // 2, js % 2
                            nc.tensor.matmul(
                                accs[t][:, 0:D + 1],
                                p_cur[a][:, sl, t * 128:(t + 1) * 128],
                                v_pair[a][:, sl, :],
                                start=(t, g) not in started and not started.add((t, g)),
                                stop=True)

            filler_sched = {
                0: list(qk_half_units(1)) + [v_unit(i) for i in range(8, 12)],
                1: [v_unit(i) for i in range(12, 16)] + list(qk_half_units(2))
                   + [v_unit(i) for i in range(16, 20)],
                2: [v_unit(i) for i in range(20, 24)] + list(qk_half_units(3))
                   + [v_unit(i) for i in range(24, 32)],
                3: [],
            }

            def finalize(g, tiles, accs, per_tile_dma=False):
                o_stage = o_pool.tile([128, 8 * (D + 1)], f32, tag="o8",
                                      name=f"o_{g}_{tiles[0]}")
                for t in tiles:
                    ch.copy_scaled(o_stage[:, t * (D + 1):(t + 1) * (D + 1)],
                                   accs[t][:, 0:D + 1], D + 1, 1.0)
                    if per_tile_dma:
                        row = (8 * g + t) * 128
                        nc.sync.dma_start(
                            out_d[row:row + 128, :],
                            o_stage[:, t * (D + 1):(t + 1) * (D + 1)])
                if per_tile_dma:
                    return
                row0 = (8 * g + tiles[0]) * 128
                n_t = len(tiles)
                nc.sync.dma_start(
                    out_d[row0:row0 + n_t * 128, :].rearrange(
                        "(t p) d -> p t d", p=128),
                    o_stage[:, tiles[0] * (D + 1):(tiles[0] + n_t) * (D + 1)
                            ].rearrange("p (t d) -> p t d", t=n_t))

            # --- upfront: bf16 fixups first (small DMAs land soonest, PE
            # warms up on them), then Q/K of group 0, V tiles 0..7 ---
            for nm_, dl_ in (("q", qT), ("k", kT)):
                if dl_[0] is None:
                    dl_[0] = qk_pool.tile([128, GQ], bf16, tag=f"{nm_}_0",
                                          name=f"{nm_}_0")

            def qk0_bf16_fix(w_sb_b, dest, nm, pool):
                ps = pool.tile([128, 512], f32,
                               tag=("psP" if pool is psP else "psA"),
                               name=f"ps0b_{nm}")
                for j in range(8):
                    nc.tensor.matmul(ps[:, 0:128], w_sb_b[:, j, :],
                                     xt0b_sb[:, j, :],
                                     start=(j == 0), stop=(j == 7))
                ch.copy_scaled(dest[0][:, 0:128], ps[:, 0:128], 128, 1.0)

            qk0_bf16_fix(wqb_sb, qT, "q", psA)
            qk0_bf16_fix(wkb_sb, kT, "k", psA)

            # bf16 V for s-tile 0 (row 0's output IS v_0 - no averaging)
            ps0v = psA.tile([128, 512], f32, tag="psA", name="ps0v")
            for j in range(8):
                nc.tensor.matmul(ps0v[:, 0:D], xt0b_sb[:, j, :],
                                 wvb_sb[:, j, :], start=(j == 0), stop=(j == 7))
            ch.copy_scaled(v0_bf[:, 0:D], ps0v[:, 0:D], D, 1.0)
            nc.gpsimd.memset(v0_bf[:, D:D + 1], 1.0)

            uf = qk_half_units(0)
            uf[0]()
            for u in uf[1:]:
                u(pool=psA)
            for i in range(8):
                v_unit(i, pool=(psA if i < 6 else psP))()

            p_cur = {}     # current group's pair tiles: a -> tile
            p_next = {}    # next group's prefetched pair tiles
            for g in range(N_G):
                n_js = 8 * g + 8
                n_pre = N_PREFETCH.get(g, 0)
                fillers = list(filler_sched[g])
                pass_tiles = PASS_TILES_G3 if g == N_G - 1 else PASS_TILES
                started = set()
                p_cur, p_next = p_next, {}

                accs1 = {}
                for t in pass_tiles[0]:
                    pool_ = psP if (g == N_G - 1 and t == 3) else psA
                    accs1[t] = pool_.tile(
                        [128, 512] if pool_ is psP else [128, D + 1], f32,
                        tag=("psP" if pool_ is psP else "psA"),
                        name=f"acc1_{g}_{t}")
                if n_pre < n_js:
                    score_exp(g, n_pre, p_cur)
                # PV for prefetched js (exp done during last group's passes)
                for js in range(n_pre):
                    if js % 2 == 1:
                        pv_emit(g, js // 2, accs1, pass_tiles[0], started,
                                "pair", p_cur)
                    else:
                        pv_emit(g, js, accs1, pass_tiles[0], started,
                                "single", p_cur)
                fill_every = max(1, (n_js - n_pre) // (len(fillers) + 1)) if fillers else 0

                def pv1(js):
                    # PV for js, lagged behind the score/exp stream so the
                    # PE's P-ldweights never waits on a just-issued exp
                    if js < n_pre or js >= n_js:
                        return
                    if js % 2 == 1:
                        pv_emit(g, js // 2, accs1, pass_tiles[0], started,
                                "pair", p_cur)
                    else:
                        pv_emit(g, js, accs1, pass_tiles[0], started,
                                "single", p_cur)

                PV_LAG = 2
                for js in range(n_pre, n_js):
                    if js != n_pre:
                        score_exp(g, js, p_cur)  # js == n_pre computed above
                    pv1(js - PV_LAG)
                    if fillers and (js - n_pre + 1) % fill_every == 0:
                        fillers.pop(0)()
                for js in range(n_js - PV_LAG, n_js):
                    pv1(js)
                for u in fillers:
                    u()
                finalize(g, pass_tiles[0], accs1)

                # PV passes 2/3 with next group's leading score/exp interleaved
                pre_next_n = 0
                n_pre_next = N_PREFETCH.get(g + 1, 0)
                for tiles in pass_tiles[1:]:
                    # cover the pass-boundary psum-reuse latency with a
                    # prefetched score matmul
                    if pre_next_n < n_pre_next:
                        score_exp(g + 1, pre_next_n, p_next)
                        pre_next_n += 1
                    accs = {}
                    for t in tiles:
                        accs[t] = psA.tile([128, D + 1], f32, tag="psA",
                                           name=f"acc_{g}_{t}")
                    for a in range(n_js // 2):
                        pv_emit(g, a, accs, tiles, started, "pair", p_cur)
                        pv_emit(g, 2 * a, accs, tiles, started, "single", p_cur)
                        if pre_next_n < n_pre_next and a % 2 == 1:
                            score_exp(g + 1, pre_next_n, p_next)
                            pre_next_n += 1
                    finalize(g, tiles, accs,
                             per_tile_dma=(g == N_G - 1 and tiles == pass_tiles[-1]))
                while pre_next_n < n_pre_next:
                    score_exp(g + 1, pre_next_n, p_next)
                    pre_next_n += 1

    nc.compile()
    return nc


def _get_nc():
    if "nc" not in _CACHE:
        _CACHE["nc"] = _build_nc()
    return _CACHE["nc"]


def _pack_xt(xb):
    """x[b] [T, C] f32 -> [N_JP*N_G, 128, 2*GQ] fp8 c-pair tiles of x^T."""
    xt = np.ascontiguousarray(xb.T)                       # [C, T] f32
    xt = xt.reshape(N_JP, 2, 128, N_G, GQ).transpose(0, 3, 2, 1, 4)
    return np.ascontiguousarray(
        xt.reshape(N_JP * N_G, 128, 2 * GQ)).astype(FP8NP)


def _pack_w(w, dtype=FP8NP, prescale=W_PRESCALE):
    """W [C, D] -> [128, 8*D]; chunk j of rows -> cols j*D:(j+1)*D."""
    wb = (w * prescale).reshape(8, 128, D).transpose(1, 0, 2)
    return np.ascontiguousarray(wb.reshape(128, 8 * D)).astype(dtype)


def _pack_xt0b(xb):
    """x[b][0:128, :]^T [C, 128] -> [128, 8*128] bf16 chunk-major."""
    xt = np.ascontiguousarray(xb[0:128, :].T)    # [C, 128]
    xt = xt.reshape(8, 128, 128).transpose(1, 0, 2)
    return np.ascontiguousarray(
        xt.reshape(128, 8 * 128)).astype(ml_dtypes.bfloat16)


def kernel(x, Wq, Wk, Wv):
    from concourse.bass_utils import run_bass_kernel_spmd

    nc = _get_nc()
    wq, wk, wv = _pack_w(Wq), _pack_w(Wk), _pack_w(Wv)
    wqb = _pack_w(Wq, ml_dtypes.bfloat16, 1.0)
    wkb = _pack_w(Wk, ml_dtypes.bfloat16, 1.0)
    wvb = _pack_w(Wv, ml_dtypes.bfloat16, 1.0)
    in_maps = []
    for b in range(N_CORES):
        in_maps.append({"xt": _pack_xt(x[b]), "wq": wq, "wk": wk, "wv": wv,
                        "xt0b": _pack_xt0b(x[b]), "wqb": wqb, "wkb": wkb,
                        "wvb": wvb})
    res = run_bass_kernel_spmd(nc, in_maps, core_ids=list(range(N_CORES)))
    outs = []
    for b in range(N_CORES):
        o = res.results[b]["out"]
        outs.append(o[:, 0:D] / o[:, D:D + 1])
    return np.stack(outs, axis=0).astype(np.float32)


# revision 50
# speedup vs baseline: 1.1844x; 1.1844x over previous
"""Single-head causal self-attention on 8 TRN2 NeuronCores.

Problem: x [8, 4096, 1024] f32, Wq/Wk/Wv [1024, 128] f32
  q/k/v = x @ W*;  out = softmax(causal(q k^T / sqrt(128))) @ v   -> [8, 4096, 128] f32

Sharding: data-parallel over batch B=8 -> one batch element per core, weights
replicated. No collectives.

Per-core plan (T=4096, C=1024, D=128):
  - x^T and W staged in fp8 (e4m3, W prescaled x64 to dodge subnormals);
    QKV projections run as fp8 DoubleRow matmuls (K=256/pass, ~1.9x bf16)
  - q/k copied to SBUF bf16 (descale 1/64); scores S^T = K^T.Q stay bf16
    (K=128 contraction gains nothing from fp8)
  - P = exp(S/sqrt(128)) written directly as fp8: ScalarE uses the Exp
    activation; the DVE share uses a Schraudolph-style one-pass
    tensor_scalar (bits = round(s*8*log2e + 55.54) -> int8 == e4m3 bits),
    accuracy ~= fp8-quantized exact exp. Work is split between ACT/DVE by
    a build-time load balancer.
  - PV accumulates acc[tq,129] over s-tile PAIRS with fp8 DoubleRow
    (V_aug pairs [128,2,129] carry a ones column -> col 128 = sum(exp));
    diagonal-adjacent odd tails run as plain fp8 matmuls
  - the softmax division happens on HOST (kernel returns [T,129] of
    numerator|denominator) - saves all on-chip reciprocal/mul work
  - rows 0..127 (tq tile 0) use a bf16 PV path: with few causal keys the
    fp8 V quantization error does not average out
  - diagonal 128x128 blocks masked on GpSimd (affine_select, s <= tq)
"""

import numpy as np
import ml_dtypes

B, T, C, D = 8, 4096, 1024, 128
N_CORES = 8
GQ = 1024              # tq group width
N_G = T // GQ          # 4 tq groups
N_JP = 4               # c-chunk pairs (C = 4 * 256)
INV_SQRT_D = 1.0 / float(np.sqrt(D))
W_PRESCALE = 64.0
LOG2E = float(np.log2(np.e))
SCH_A = 8.0 * LOG2E * INV_SQRT_D   # schraudolph scale on raw score psum
SCH_B = 56.0 - 0.46                # fitted offset (DVE convert rounds-to-nearest)
FP8NP = ml_dtypes.float8_e4m3

PASS_TILES = [(0, 1, 2), (3, 4, 5), (6, 7)]
PASS_TILES_G3 = [(0, 1, 2, 3), (4, 5), (6, 7)]
N_PREFETCH = {1: 2, 2: 5, 3: 12}   # js of group g prefetched into g-1's passes

_CACHE = {}

# build-time schedule options (A/B-tunable; _CACHE is keyed by these)
OPTS = {
    "exp_alt": False,   # strict ACT/DVE alternation for exp (vs busy-based)
    "fill_mode": "subop",  # "subop": interleave filler matmuls; "unit": whole units
    "pv_lag": 3,        # js lag between exp and its PV matmuls
    "warmup": 12,       # dummy matmuls while input DMAs land (ramps PE p-state)
    "score_cols": 512,  # moving width of score matmuls (512 or 1024)
    "prefetch": (4, 8, 16),  # js of groups 1..3 prefetched into prior passes
    "g3_alt_order": True,   # g3 passes (0123)(67)(45) instead of (0123)(45)(67)
    "g3_all_stream": False,  # per-tile output DMA for all g3 passes
}


class Chooser:
    """Build-time ACT/DVE load balancer for psum-reading elementwise ops."""

    def __init__(self, nc, mybir):
        self.nc = nc
        self.mybir = mybir
        self.busy = {"act": 0.0, "dve": 0.0}
        self.exp_i = 0

    def _pick(self, cols, act_extra=0.0):
        ca = self.busy["act"] + 0.833 * cols + 260 + act_extra
        cd = self.busy["dve"] + 1.042 * cols + 145
        return "act" if ca <= cd else "dve"

    def exp_fp8(self, out_ap, in_ap, cols):
        if OPTS["exp_alt"]:
            # strict alternation: exp(js) and exp(js+1) always land on
            # different engines, so psS double-buffering never waits on a burst
            eng = "act" if self.exp_i % 2 == 0 else "dve"
            self.exp_i += 1
        else:
            eng = self._pick(cols)
        if eng == "act":
            self.nc.scalar.activation(out_ap, in_ap,
                                      self.mybir.ActivationFunctionType.Exp,
                                      scale=INV_SQRT_D)
            self.busy["act"] += 0.833 * cols + 260
        else:
            self.nc.vector.tensor_scalar(out_ap.bitcast(self.mybir.dt.int8),
                                         in_ap, SCH_A, SCH_B,
                                         self.mybir.AluOpType.mult,
                                         self.mybir.AluOpType.add)
            self.busy["dve"] += 1.042 * cols + 145
        return eng

    def copy_scaled(self, out_ap, in_ap, cols, scale):
        eng = self._pick(cols)
        if eng == "act":
            self.nc.scalar.activation(out_ap, in_ap,
                                      self.mybir.ActivationFunctionType.Copy,
                                      scale=scale)
            self.busy["act"] += 0.833 * cols + 260
        else:
            self.nc.vector.tensor_scalar(out_ap, in_ap, scale, None,
                                         self.mybir.AluOpType.mult)
            self.busy["dve"] += 1.042 * cols + 145
        return eng


def _build_nc():
    import concourse.tile as tile
    from concourse import bacc, mybir

    f32 = mybir.dt.float32
    bf16 = mybir.dt.bfloat16
    fp8 = mybir.dt.float8e4
    DR = mybir.MatmulPerfMode.DoubleRow

    nc = bacc.Bacc(None, target_bir_lowering=False)
    # xt[jp*N_G + g] is [128, 2*GQ]: c-pair jp, group g, slots (2jp, 2jp+1)
    xt_d = nc.declare_dram_parameter("xt", [N_JP * N_G, 128, 2 * GQ], fp8,
                                     isOutput=False)
    wq_d = nc.declare_dram_parameter("wq", [128, 8 * D], fp8, isOutput=False)
    wk_d = nc.declare_dram_parameter("wk", [128, 8 * D], fp8, isOutput=False)
    wv_d = nc.declare_dram_parameter("wv", [128, 8 * D], fp8, isOutput=False)
    # bf16 copies for the first 128 rows of Q^T/K^T: with few causal keys the
    # fp8 projection noise does not average out of the softmax
    xt0b_d = nc.declare_dram_parameter("xt0b", [128, 8 * 128], bf16,
                                       isOutput=False)
    wqb_d = nc.declare_dram_parameter("wqb", [128, 8 * D], bf16, isOutput=False)
    wkb_d = nc.declare_dram_parameter("wkb", [128, 8 * D], bf16, isOutput=False)
    wvb_d = nc.declare_dram_parameter("wvb", [128, 8 * D], bf16, isOutput=False)
    out_d = nc.declare_dram_parameter("out", [T, D + 1], f32, isOutput=True)

    with tile.TileContext(nc) as tc:
        with (
            tc.tile_pool(name="consts", bufs=1) as consts,
            tc.tile_pool(name="xt", bufs=1) as xt_pool,
            tc.tile_pool(name="qk", bufs=1) as qk_pool,
            tc.tile_pool(name="vaug", bufs=1) as vaug_pool,
            tc.tile_pool(name="p", bufs=1) as p_pool,
            tc.tile_pool(name="osb", bufs=2) as o_pool,
            tc.tile_pool(name="psS", bufs=2, space="PSUM") as psS,
            tc.tile_pool(name="psA", bufs=3, space="PSUM") as psA,
            tc.tile_pool(name="psP", bufs=1, space="PSUM") as psP,
        ):
            ch = Chooser(nc, mybir)

            wq_sb = consts.tile([128, 8, D], fp8, tag="wq")
            wk_sb = consts.tile([128, 8, D], fp8, tag="wk")
            wv_sb = consts.tile([128, 8, D], fp8, tag="wv")

            xt_sb = [[None] * N_G for _ in range(N_JP)]

            def xt_dma(jp, g, eng=None):
                t_ = xt_pool.tile([128, 2, GQ], fp8, tag=f"xt_{jp}_{g}",
                                  name=f"xt_{jp}_{g}")
                (eng or nc.sync).dma_start(t_[:].rearrange("p a b -> p (a b)"),
                                           xt_d[jp * N_G + g])
                xt_sb[jp][g] = t_

            xt0b_sb = consts.tile([128, 8, 128], bf16, tag="xt0b")
            wqb_sb = consts.tile([128, 8, D], bf16, tag="wqb")
            wkb_sb = consts.tile([128, 8, D], bf16, tag="wkb")
            wvb_sb = consts.tile([128, 8, D], bf16, tag="wvb")

            # startup DMAs fan out over four engine queues so group-0 inputs
            # land ~4x sooner; bf16 fixup inputs go first (PE warm-up work)
            nc.sync.dma_start(wqb_sb[:].rearrange("p a b -> p (a b)"), wqb_d[:])
            nc.scalar.dma_start(wkb_sb[:].rearrange("p a b -> p (a b)"), wkb_d[:])
            nc.gpsimd.dma_start(xt0b_sb[:].rearrange("p a b -> p (a b)"), xt0b_d[:])
            xt_dma(2, 0, nc.sync)
            nc.scalar.dma_start(wq_sb[:].rearrange("p a b -> p (a b)"), wq_d[:])
            nc.gpsimd.dma_start(wk_sb[:].rearrange("p a b -> p (a b)"), wk_d[:])
            nc.sync.dma_start(wvb_sb[:].rearrange("p a b -> p (a b)"), wvb_d[:])
            xt_dma(0, 0, nc.scalar)
            xt_dma(1, 0, nc.gpsimd)
            nc.sync.dma_start(wv_sb[:].rearrange("p a b -> p (a b)"), wv_d[:])
            xt_dma(3, 0, nc.scalar)
            for g in range(1, N_G):
                for jp in range(N_JP):
                    xt_dma(jp, g)

            qT = [None] * N_G          # [128, GQ] bf16
            kT = [None] * N_G
            v_pair = [None] * (T // 256)   # [128, 2, D+1] fp8, ones col at 128
            v0_bf = consts.tile([128, D + 1], bf16, tag="v0bf")
            p00_bf = consts.tile([128, 128], bf16, tag="p00bf")

            def qk_half_subops(g, nm, h, w_sb, dest_list):
                """Q^T/K^T half for group g as 5 sub-ops (4 DR matmuls + copy)
                so the filler scheduler can interleave them between score
                matmuls and hide their ldweights.

                For g==0 h==0 only cols 128:512 are produced - cols 0:128 come
                from the bf16 fixup units (which run first)."""
                holder = {}
                lo = 128 if (g == 0 and h == 0) else 0
                hs = slice(h * 512 + lo, (h + 1) * 512)
                n_cols = 512 - lo

                def mm(jp, pool=None):
                    if "ps" not in holder:
                        pp = pool if pool is not None else psP
                        holder["ps"] = pp.tile(
                            [128, 512], f32,
                            tag=("psP" if pp is psP else "psA"),
                            name=f"ps{nm}_{g}_{h}")
                    nc.tensor.matmul(
                        holder["ps"][:, 0:n_cols], w_sb[:, 2 * jp:2 * jp + 2, :],
                        xt_sb[jp][g][:, :, hs],
                        start=(jp == 0), stop=(jp == N_JP - 1), perf_mode=DR)

                def cp(pool=None):
                    if dest_list[g] is None:
                        dest_list[g] = qk_pool.tile([128, GQ], bf16,
                                                    tag=f"{nm}_{g}",
                                                    name=f"{nm}_{g}")
                    ch.copy_scaled(dest_list[g][:, hs], holder["ps"][:, 0:n_cols],
                                   n_cols, 1.0 / W_PRESCALE)

                return [lambda pool=None, jp=jp: mm(jp, pool)
                        for jp in range(N_JP)] + [cp]

            def qk_units(g):
                subs = []
                for w_sb, dest_list, nm in ((wq_sb, qT, "q"), (wk_sb, kT, "k")):
                    for h in range(2):
                        subs.append(qk_half_subops(g, nm, h, w_sb, dest_list))
                return subs

            def v_subops(i):
                g, off = i // 8, (i % 8) * 128
                holder = {}

                def mm(jp, pool=None):
                    if "ps" not in holder:
                        pp = pool if pool is not None else psP
                        holder["ps"] = pp.tile(
                            [128, 512], f32,
                            tag=("psP" if pp is psP else "psA"),
                            name=f"psv_{i}")
                    nc.tensor.matmul(
                        holder["ps"][:, 0:D], xt_sb[jp][g][:, :, off:off + 128],
                        wv_sb[:, 2 * jp:2 * jp + 2, :],
                        start=(jp == 0), stop=(jp == N_JP - 1), perf_mode=DR)

                def cp(pool=None):
                    a, sl = i // 2, i % 2
                    if v_pair[a] is None:
                        v_pair[a] = vaug_pool.tile([128, 2, D + 1], fp8,
                                                   tag=f"v_{a}", name=f"v_{a}")
                    ch.copy_scaled(v_pair[a][:, sl, 0:D], holder["ps"][:, 0:D],
                                   D, 1.0 / W_PRESCALE)
                    nc.gpsimd.memset(v_pair[a][:, sl, D:D + 1], 1.0)

                return [lambda pool=None, jp=jp: mm(jp, pool)
                        for jp in range(N_JP)] + [cp]

            def run_unit(subs, pool=None):
                for s in subs:
                    s(pool=pool)

            def score_exp(g, js, dest, fill=None):
                """dest: per-group dict a -> [128, 2, GQ] fp8 tile.
                fill: optional callback popping filler sub-ops between the
                score matmuls (their ldweights hide under the score stream)."""
                off = max(0, (js - 8 * g)) * 128
                pss = psS.tile([128, GQ], f32, tag="psS", name=f"pss_{g}_{js}")
                sc = OPTS["score_cols"]
                for h in range(GQ // sc):
                    lo = max(off, h * sc)
                    if (h + 1) * sc > lo:
                        nc.tensor.matmul(pss[:, lo:(h + 1) * sc],
                                         kT[js // 8][:, (js % 8) * 128:(js % 8 + 1) * 128],
                                         qT[g][:, lo:(h + 1) * sc],
                                         start=True, stop=True)
                    if fill is not None:
                        fill()
                a, sl = js // 2, js % 2
                if a not in dest:
                    dest[a] = p_pool.tile([128, 2, GQ], fp8, tag=f"p_{a}",
                                          bufs=(2 if a < 10 else 1),
                                          name=f"p_{g}_{a}")
                p_t = dest[a]
                ch.exp_fp8(p_t[:, sl, off:GQ], pss[:, off:GQ], GQ - off)
                if js >= 8 * g:
                    nc.gpsimd.affine_select(
                        out=p_t[:, sl, off:off + 128].bitcast(mybir.dt.int8),
                        in_=p_t[:, sl, off:off + 128].bitcast(mybir.dt.int8),
                        compare_op=mybir.AluOpType.is_ge,
                        fill=0.0, base=0, pattern=[[1, 128]],
                        channel_multiplier=-1)
                if g == 0 and js == 0:
                    # bf16 P for rows 0..127 (tile-0 special case)
                    nc.scalar.activation(p00_bf[:], pss[:, 0:128],
                                         mybir.ActivationFunctionType.Exp,
                                         scale=INV_SQRT_D)
                    ch.busy["act"] += 0.833 * 128 + 260
                    nc.gpsimd.affine_select(
                        out=p00_bf[:], in_=p00_bf[:],
                        compare_op=mybir.AluOpType.is_ge,
                        fill=0.0, base=0, pattern=[[1, 128]],
                        channel_multiplier=-1)
                return p_t

            def pv_emit(g, a_or_js, accs, tiles, started, mode, p_cur):
                """Emit PV matmuls for pair index a (mode='pair': both slots
                valid) or diagonal single js (mode='single')."""
                for t in tiles:
                    T_t = 8 * g + t
                    if mode == "pair":
                        a = a_or_js
                        if 2 * a + 1 > T_t:
                            continue
                        last = (T_t - 1) // 2 == a and T_t % 2 == 1
                        if g == 0 and t == 0:
                            continue  # tile 0 handled by bf16 special case
                        nc.tensor.matmul(
                            accs[t][:, 0:D + 1],
                            p_cur[a][:, :, t * 128:(t + 1) * 128],
                            v_pair[a][:],
                            start=(t, g) not in started and not started.add((t, g)),
                            stop=last, perf_mode=DR)
                    else:
                        js = a_or_js
                        if T_t != js:
                            continue
                        if g == 0 and t == 0:
                            nc.tensor.matmul(accs[t][:, 0:D + 1], p00_bf[:],
                                             v0_bf[:], start=True, stop=True)
                        else:
                            a, sl = js // 2, js % 2
                            nc.tensor.matmul(
                                accs[t][:, 0:D + 1],
                                p_cur[a][:, sl, t * 128:(t + 1) * 128],
                                v_pair[a][:, sl, :],
                                start=(t, g) not in started and not started.add((t, g)),
                                stop=True)

            filler_sched = {
                0: qk_units(1) + [v_subops(i) for i in range(8, 12)],
                1: [v_subops(i) for i in range(12, 16)] + qk_units(2)
                   + [v_subops(i) for i in range(16, 20)],
                2: [v_subops(i) for i in range(20, 24)] + qk_units(3)
                   + [v_subops(i) for i in range(24, 32)],
                3: [],
            }

            def finalize(g, tiles, accs, per_tile_dma=False):
                o_stage = o_pool.tile([128, 8 * (D + 1)], f32, tag="o8",
                                      name=f"o_{g}_{tiles[0]}")
                for t in tiles:
                    ch.copy_scaled(o_stage[:, t * (D + 1):(t + 1) * (D + 1)],
                                   accs[t][:, 0:D + 1], D + 1, 1.0)
                    if per_tile_dma:
                        row = (8 * g + t) * 128
                        nc.sync.dma_start(
                            out_d[row:row + 128, :],
                            o_stage[:, t * (D + 1):(t + 1) * (D + 1)])
                if per_tile_dma:
                    return
                row0 = (8 * g + tiles[0]) * 128
                n_t = len(tiles)
                nc.sync.dma_start(
                    out_d[row0:row0 + n_t * 128, :].rearrange(
                        "(t p) d -> p t d", p=128),
                    o_stage[:, tiles[0] * (D + 1):(tiles[0] + n_t) * (D + 1)
                            ].rearrange("p (t d) -> p t d", t=n_t))

            # --- warmup: dummy matmuls on a zeroed tile while the input DMAs
            # land; soaks the startup bubble and ramps the PE p-state ---
            if OPTS["warmup"]:
                warm = consts.tile([128, 512], bf16, tag="warm")
                nc.vector.memset(warm[:], 0.0)  # vector is idle earliest
                for wi in range(OPTS["warmup"]):
                    psw = psP.tile([128, 512], f32, tag="psP", name=f"warm_{wi}")
                    nc.tensor.matmul(psw[:], warm[:, 0:128], warm[:],
                                     start=True, stop=True)

            # --- upfront: bf16 fixups first (small DMAs land soonest, PE
            # warms up on them), then Q/K of group 0, V tiles 0..7 ---
            for nm_, dl_ in (("q", qT), ("k", kT)):
                if dl_[0] is None:
                    dl_[0] = qk_pool.tile([128, GQ], bf16, tag=f"{nm_}_0",
                                          name=f"{nm_}_0")

            def qk0_bf16_fix(w_sb_b, dest, nm, pool):
                ps = pool.tile([128, 512], f32,
                               tag=("psP" if pool is psP else "psA"),
                               name=f"ps0b_{nm}")
                for j in range(8):
                    nc.tensor.matmul(ps[:, 0:128], w_sb_b[:, j, :],
                                     xt0b_sb[:, j, :],
                                     start=(j == 0), stop=(j == 7))
                ch.copy_scaled(dest[0][:, 0:128], ps[:, 0:128], 128, 1.0)

            qk0_bf16_fix(wqb_sb, qT, "q", psA)
            qk0_bf16_fix(wkb_sb, kT, "k", psA)

            # bf16 V for s-tile 0 (row 0's output IS v_0 - no averaging)
            ps0v = psA.tile([128, 512], f32, tag="psA", name="ps0v")
            for j in range(8):
                nc.tensor.matmul(ps0v[:, 0:D], xt0b_sb[:, j, :],
                                 wvb_sb[:, j, :], start=(j == 0), stop=(j == 7))
            ch.copy_scaled(v0_bf[:, 0:D], ps0v[:, 0:D], D, 1.0)
            nc.gpsimd.memset(v0_bf[:, D:D + 1], 1.0)

            uf = qk_units(0)
            run_unit(uf[0])
            for u in uf[1:]:
                run_unit(u, pool=psA)
            for i in range(8):
                run_unit(v_subops(i), pool=(psA if i < 6 else psP))

            p_cur = {}     # current group's pair tiles: a -> tile
            p_next = {}    # next group's prefetched pair tiles
            for g in range(N_G):
                n_js = 8 * g + 8
                n_pre = OPTS["prefetch"][g - 1] if g >= 1 else 0
                fillers = [s for unit in filler_sched[g] for s in unit]
                if g == N_G - 1:
                    pass_tiles = ([(0, 1, 2, 3), (6, 7), (4, 5)]
                                  if OPTS["g3_alt_order"] else PASS_TILES_G3)
                else:
                    pass_tiles = PASS_TILES
                started = set()
                p_cur, p_next = p_next, {}

                accs1 = {}
                for t in pass_tiles[0]:
                    pool_ = psP if (g == N_G - 1 and t == 3) else psA
                    accs1[t] = pool_.tile(
                        [128, 512] if pool_ is psP else [128, D + 1], f32,
                        tag=("psP" if pool_ is psP else "psA"),
                        name=f"acc1_{g}_{t}")
                if n_pre < n_js:
                    score_exp(g, n_pre, p_cur)
                # PV for prefetched js (exp done during last group's passes)
                for js in range(n_pre):
                    if js % 2 == 1:
                        pv_emit(g, js // 2, accs1, pass_tiles[0], started,
                                "pair", p_cur)
                    else:
                        pv_emit(g, js, accs1, pass_tiles[0], started,
                                "single", p_cur)
                def pv1(js):
                    # PV for js, lagged behind the score/exp stream so the
                    # PE's P-ldweights never waits on a just-issued exp
                    if js < n_pre or js >= n_js:
                        return
                    if js % 2 == 1:
                        pv_emit(g, js // 2, accs1, pass_tiles[0], started,
                                "pair", p_cur)
                    else:
                        pv_emit(g, js, accs1, pass_tiles[0], started,
                                "single", p_cur)

                fstate = {"i": 0, "target": 0.0}
                n_slots = max(1, n_js - n_pre)
                f_rate = len(fillers) / n_slots
                subop_mode = OPTS["fill_mode"] == "subop"

                def pop_fill():
                    if fstate["i"] < min(len(fillers), int(fstate["target"])):
                        fillers[fstate["i"]]()
                        fstate["i"] += 1

                PV_LAG = OPTS["pv_lag"]
                for js in range(n_pre, n_js):
                    fstate["target"] += f_rate
                    if js != n_pre:
                        score_exp(g, js, p_cur,
                                  fill=(pop_fill if subop_mode else None))
                    pv1(js - PV_LAG)
                    if subop_mode:
                        pop_fill()
                        pop_fill()
                        pop_fill()
                    else:
                        while fstate["i"] < min(len(fillers),
                                                int(fstate["target"])):
                            fillers[fstate["i"]]()
                            fstate["i"] += 1
                for js in range(n_js - PV_LAG, n_js):
                    pv1(js)
                while fstate["i"] < len(fillers):
                    fillers[fstate["i"]]()
                    fstate["i"] += 1
                finalize(g, pass_tiles[0], accs1)

                # PV passes 2/3 with next group's leading score/exp interleaved
                pre_next_n = 0
                n_pre_next = (OPTS["prefetch"][g] if g + 1 < N_G
                              else 0)
                for tiles in pass_tiles[1:]:
                    # cover the pass-boundary psum-reuse latency with a
                    # prefetched score matmul
                    if pre_next_n < n_pre_next:
                        score_exp(g + 1, pre_next_n, p_next)
                        pre_next_n += 1
                    accs = {}
                    for t in tiles:
                        accs[t] = psA.tile([128, D + 1], f32, tag="psA",
                                           name=f"acc_{g}_{t}")
                    for a in range(n_js // 2):
                        pv_emit(g, a, accs, tiles, started, "pair", p_cur)
                        pv_emit(g, 2 * a, accs, tiles, started, "single", p_cur)
                        if pre_next_n < n_pre_next and a % 2 == 1:
                            score_exp(g + 1, pre_next_n, p_next)
                            pre_next_n += 1
                    finalize(g, tiles, accs,
                             per_tile_dma=(g == N_G - 1 and
                                           (OPTS["g3_all_stream"] or
                                            tiles == pass_tiles[-1])))
                while pre_next_n < n_pre_next:
                    score_exp(g + 1, pre_next_n, p_next)
                    pre_next_n += 1

    nc.compile()
    return nc


def _get_nc():
    key = tuple((k, tuple(v) if isinstance(v, (list, tuple)) else v)
                for k, v in sorted(OPTS.items()))
    if key not in _CACHE:
        _CACHE[key] = _build_nc()
    return _CACHE[key]


def _pack_xt(xb):
    """x[b] [T, C] f32 -> [N_JP*N_G, 128, 2*GQ] fp8 c-pair tiles of x^T."""
    xt = np.ascontiguousarray(xb.T)                       # [C, T] f32
    xt = xt.reshape(N_JP, 2, 128, N_G, GQ).transpose(0, 3, 2, 1, 4)
    return np.ascontiguousarray(
        xt.reshape(N_JP * N_G, 128, 2 * GQ)).astype(FP8NP)


def _pack_w(w, dtype=FP8NP, prescale=W_PRESCALE):
    """W [C, D] -> [128, 8*D]; chunk j of rows -> cols j*D:(j+1)*D."""
    wb = (w * prescale).reshape(8, 128, D).transpose(1, 0, 2)
    return np.ascontiguousarray(wb.reshape(128, 8 * D)).astype(dtype)


def _pack_xt0b(xb):
    """x[b][0:128, :]^T [C, 128] -> [128, 8*128] bf16 chunk-major."""
    xt = np.ascontiguousarray(xb[0:128, :].T)    # [C, 128]
    xt = xt.reshape(8, 128, 128).transpose(1, 0, 2)
    return np.ascontiguousarray(
        xt.reshape(128, 8 * 128)).astype(ml_dtypes.bfloat16)


def kernel(x, Wq, Wk, Wv):
    from concourse.bass_utils import run_bass_kernel_spmd

    nc = _get_nc()
    wq, wk, wv = _pack_w(Wq), _pack_w(Wk), _pack_w(Wv)
    wqb = _pack_w(Wq, ml_dtypes.bfloat16, 1.0)
    wkb = _pack_w(Wk, ml_dtypes.bfloat16, 1.0)
    wvb = _pack_w(Wv, ml_dtypes.bfloat16, 1.0)
    in_maps = []
    for b in range(N_CORES):
        in_maps.append({"xt": _pack_xt(x[b]), "wq": wq, "wk": wk, "wv": wv,
                        "xt0b": _pack_xt0b(x[b]), "wqb": wqb, "wkb": wkb,
                        "wvb": wvb})
    res = run_bass_kernel_spmd(nc, in_maps, core_ids=list(range(N_CORES)))
    outs = []
    for b in range(N_CORES):
        o = res.results[b]["out"]
        outs.append(o[:, 0:D] / o[:, D:D + 1])
    return np.stack(outs, axis=0).astype(np.float32)
